# revision 18
# baseline (speedup 1.0000x reference)
"""GAT layer kernel for Trainium2, sharded across 8 NeuronCores.

Math: reference computes
    h = x @ W.T;  e_ij = (h @ a1)[i] + (h @ a2)[j];  mask by adj;
    softmax over j; out = attn @ h.
Because e_i is constant along the softmax axis it cancels, so with
w_j = exp(h_j . a2):
    out[i] = sum_j adj[i,j] * w_j * h[j] / sum_j adj[i,j] * w_j
a1 is mathematically irrelevant.

Split of work:
  host:   w = exp(x @ (W.T a2))  (4 MFLOP) and the exact denominator
          den = adj @ w (a 134 MFLOP BLAS matvec); final division and
          transpose of the gathered numerator.
  device: h = x @ W (bf16), hw = e4m3(w * h), and the big numerator
          num.T = sum_j hw[j,:] outer adj[j,:]  as a DoubleRow fp8
          matmul with hw STATIONARY and the adjacency MOVING
          (free dim 512, 2x contraction per pass, LDWEIGHTS hidden).

Sharding: rows of adj (and of the output) are split across 8 cores; h
is small so every core computes the full h redundantly.

dtype strategy: adj is 0/1 so it is EXACT in fp8 e4m3 -- the host emits
the e4m3 bit pattern directly (0x00/0x38): the 67 MB adjacency loads as
1 byte/element with no cast anywhere.  hw is a single product
quantization to e4m3 (measured end-to-end rel err ~1.0e-2 vs the 2e-2
budget).  PSUM: 4 banks hold the 4 accumulators [128,512], 4 banks
double-buffer phase-1 h tiles, so both phases interleave on the PE.
"""

import sys

import numpy as np
import ml_dtypes

for _p in ("/opt/trn_rl_repo",):
    try:
        import concourse.bass  # noqa: F401

        break
    except ImportError:
        if _p not in sys.path:
            sys.path.insert(0, _p)

import concourse.bass as bass
import concourse.mybir as mybir
import concourse.tile as tile
from concourse.bass_utils import run_bass_kernel_spmd

dt = mybir.dt
AF = mybir.ActivationFunctionType
PM = mybir.MatmulPerfMode

N = 8192
D = 256
NCORES = 8
RB = N // NCORES  # 1024 output rows per core
NJ = N // 128  # 64 j-chunks
NG = NJ // 2  # 32 DoubleRow pair-groups
NI = D // 128  # 2 contraction chunks for h

# ---------------------------------------------------------------------------
# walrus in this container accepts at most ONE sync-wait command on several
# instruction structs (Drain, 4-byte self-loading Matmult, ...) while the
# newer Tile scheduler emits more. Split the extras into single-wait
# EventSemaphore prefixes on the same engine (identical semantics).
_ev_counter = [0]


def _legalize_multiwait(nc, max_keep=1):
    for f in nc.m.functions:
        for bb in f.blocks:
            il = bb.instructions
            idx = 0
            while idx < len(il):
                inst = il[idx]
                si = inst.sync_info
                if si is not None and si.on_wait and len(si.on_wait) > max_keep:
                    waits = list(si.on_wait)
                    keep = waits[len(waits) - max_keep :] if max_keep else []
                    extra = waits[: len(waits) - max_keep] if max_keep else waits
                    si.on_wait = keep
                    for w in extra:
                        _ev_counter[0] += 1
                        ev = mybir.InstEventSemaphore(
                            name=f"lgw_{_ev_counter[0]}", ins=[], outs=[]
                        )
                        ev.engine = inst.engine
                        ev.sync_info = mybir.SyncInfo(on_wait=[w], on_update=[])
                        il.insert(idx, ev)
                        idx += 1
                idx += 1


# ---------------------------------------------------------------------------


def _build_program():
    nc = bass.Bass("TRN2", debug=False)

    xT = nc.dram_tensor("xT", [D, N], dt.bfloat16, kind="ExternalInput").ap()
    WT = nc.dram_tensor("WT", [D, D], dt.bfloat16, kind="ExternalInput").ap()
    # wcol[p, jc] = w[128*jc + p]
    wcol = nc.dram_tensor("wcol", [128, NJ], dt.float32, kind="ExternalInput").ap()
    # adjM[p, jg, i, r] = adj[this core's row r, col 256*jg + 128*i + p], e4m3
    adjM = nc.dram_tensor(
        "adjM", [128, NG, 2, RB], dt.float8e4, kind="ExternalInput"
    ).ap()
    # numerator, k-major: outT[k, r]
    outT = nc.dram_tensor("outT", [D, RB], dt.float32, kind="ExternalOutput").ap()

    XCH = 1024  # x streamed in [128, XCH] bf16 chunks (256 KB per DMA)
    NXB = N // XCH
    NCPB = XCH // 128

    with tile.TileContext(nc) as tc:
        with (
            tc.tile_pool(name="xr", bufs=1) as xr_pool,
            tc.tile_pool(name="wte", bufs=1) as wte_pool,
            tc.tile_pool(name="wc", bufs=1) as wc_pool,
            tc.tile_pool(name="hw", bufs=1) as hw_pool,
            tc.tile_pool(name="adjr", bufs=6) as adj_pool,
            tc.tile_pool(name="outs", bufs=4) as out_pool,
            tc.tile_pool(name="dmy", bufs=1) as dmy_pool,
        ):
            # ---- params + x stream on the SWDGE (gpsimd) queue: keeps the
            # ACT sequencer free for phase-1 drains (HWDGE descriptor gen is
            # ~0.6us *serial* on the issuing engine) and gives the x stream
            # its own DMA queue beside the adjacency's SP ring.  The first
            # x pair goes out before everything else: it gates the first
            # matmul while the adjacency prefetch floods HBM.
            xr = [[None] * NXB for _ in range(NI)]
            for b in range(NXB):
                for ic in range(NI):
                    xr[ic][b] = xr_pool.tile(
                        [128, XCH], dt.bfloat16, name=f"xr{ic}_{b}", tag="x", bufs=8
                    )

            def _dma_x(b, eng):
                for ic in range(NI):
                    eng.dma_start(
                        xr[ic][b],
                        xT[ic * 128 : (ic + 1) * 128, b * XCH : (b + 1) * XCH],
                    )

            # the b=0 pair rides the SP HWDGE ring AHEAD of the adjacency
            # stream (FIFO per ring), so it lands at line rate ~3.5us in.
            # tiny param loads ride the otherwise-idle ACT ring so they land
            # ~2us in; the SWDGE queue takes ~4us to its first completion.
            wc = wc_pool.tile([128, NJ], dt.float32, name="wc")
            nc.scalar.dma_start(wc, wcol)
            wte = []
            for ic in range(NI):
                t = wte_pool.tile([128, D], dt.bfloat16, name=f"wte{ic}")
                nc.scalar.dma_start(t, WT[ic * 128 : (ic + 1) * 128, :])
                wte.append(t)
            _dma_x(0, nc.sync)
            for b in range(1, NXB):
                _dma_x(b, nc.gpsimd)

            hw_all = hw_pool.tile([128, NJ, D], dt.float8e4, name="hw_all")

            with (
                tc.tile_pool(name="ph", bufs=4, space="PSUM") as ph_pool,
                tc.tile_pool(name="acc", bufs=1, space="PSUM") as acc_pool,
            ):
                acc = [
                    acc_pool.tile([128, 512], dt.float32, name=f"acc{kt}_{rh}")
                    for kt in range(2)
                    for rh in range(2)
                ]
                # ---- HAM warmup: ~2.1us of zero matmuls with no data deps
                # so the PE clock-gate opens before the real stream begins.
                dmy = dmy_pool.tile([128, 256], dt.bfloat16, name="dmy")
                nc.vector.memset(dmy, 0)
                for k in range(12):
                    ph = ph_pool.tile([128, D], dt.float32, name="ph", tag="ph")
                    nc.tensor.matmul(ph, dmy[:, 0:128], dmy, start=True, stop=True)

                def phase1_pair(g):
                    # h matmuls + scaled fp8 drains for j-chunks 2g, 2g+1
                    for i in range(2):
                        jc = 2 * g + i
                        b, sl = jc // NCPB, bass.ts(jc % NCPB, 128)
                        ph = ph_pool.tile([128, D], dt.float32, name="ph", tag="ph")
                        nc.tensor.matmul(
                            ph, xr[0][b][:, sl], wte[0], start=True, stop=False
                        )
                        nc.tensor.matmul(
                            ph, xr[1][b][:, sl], wte[1], start=False, stop=True
                        )
                        # scaled drain: hw[jc] = e4m3(w_j * h_j)
                        wv = wc[:, jc : jc + 1]
                        if i == 0:
                            nc.vector.tensor_scalar_mul(hw_all[:, jc, :], ph, wv)
                        else:
                            nc.scalar.activation(
                                hw_all[:, jc, :], ph, AF.Copy, scale=wv
                            )

                # software pipeline: phase 1 runs one pair-group ahead, so
                # the DoubleRow matmuls never wait on a fresh drain.
                phase1_pair(0)
                for jg in range(NG):
                    at = adj_pool.tile([128, 2, RB], dt.float8e4, name="at", tag="at")
                    nc.sync.dma_start(at, adjM[:, jg, :, :])
                    if jg + 1 < NG:
                        phase1_pair(jg + 1)
                    # phase 2: num.T += hw_pair.T @ adj_pair  (DoubleRow fp8)
                    for kt in range(2):
                        lhs = hw_all[:, 2 * jg : 2 * jg + 2, kt * 128 : (kt + 1) * 128]
                        for rh in range(2):
                            nc.tensor.matmul(
                                acc[2 * kt + rh],
                                lhs,
                                at[:, :, rh * 512 : (rh + 1) * 512],
                                perf_mode=PM.DoubleRow,
                                start=(jg == 0),
                                stop=(jg == NG - 1),
                                skip_group_check=True,
                            )

                # ---- epilogue: drain the 4 accumulators, store num.T.
                # Half-tile copies fan out over DVE/ACT/GpSimd (all idle
                # now); stores split across both HWDGE rings (the SP ring
                # is idle once the adj stream is done).
                cpy = [
                    nc.vector.tensor_copy,
                    lambda o, i_: nc.scalar.activation(o, i_, AF.Copy),
                ]
                ci = 0
                for kt in range(2):
                    for rh in range(2):
                        ob = out_pool.tile([128, 512], dt.float32, name="ob", tag="ob")
                        for hf in range(2):
                            sl = slice(hf * 256, (hf + 1) * 256)
                            cpy[ci % 2](ob[:, sl], acc[2 * kt + rh][:, sl])
                            ci += 1
                        eng = nc.sync if rh == 0 else nc.scalar
                        eng.dma_start(
                            outT[
                                kt * 128 : (kt + 1) * 128,
                                rh * 512 : (rh + 1) * 512,
                            ],
                            ob,
                        )

    _legalize_multiwait(nc, max_keep=1)
    return nc


_CACHED = {}


def _prep_inputs(x, adj, W, a):
    xT = np.ascontiguousarray(x.T).astype(ml_dtypes.bfloat16)
    WT = np.ascontiguousarray(W.T).astype(ml_dtypes.bfloat16)

    wa2 = W.T.astype(np.float64) @ a[D:].astype(np.float64)
    e_host = (x.astype(np.float64) @ wa2).astype(np.float32)
    w_host = np.exp(e_host)  # [N] f32
    wcol = np.ascontiguousarray(w_host.reshape(NJ, 128).T)  # [128, NJ]

    in_maps = []
    dens = []
    for c in range(NCORES):
        blk = adj[c * RB : (c + 1) * RB, :]  # [RB, N] int32
        bits = (blk.T != 0).astype(np.uint8) * np.uint8(0x38)  # [N, RB]
        adjM = np.ascontiguousarray(
            bits.reshape(NG, 2, 128, RB).transpose(2, 0, 1, 3)
        ).view(ml_dtypes.float8_e4m3)
        dens.append(blk.astype(np.float32) @ w_host)  # exact denominator
        in_maps.append({"xT": xT, "WT": WT, "wcol": wcol, "adjM": adjM})
    return in_maps, dens


def _run(in_maps, **kw):
    if "nc" not in _CACHED:
        _CACHED["nc"] = _build_program()
    # The device occasionally comes up wedged (NRT_EXEC_UNIT_UNRECOVERABLE)
    # from a previous process; one retry after a short pause recovers it.
    import time as _time

    last_err = None
    for attempt in range(3):
        try:
            return run_bass_kernel_spmd(
                _CACHED["nc"], in_maps, core_ids=list(range(NCORES)), **kw
            )
        except Exception as e:  # noqa: BLE001
            last_err = e
            if "UNRECOVERABLE" not in str(e) and "UNAVAILABLE" not in str(e):
                raise
            _time.sleep(3.0)
    raise last_err


def _finish(res, dens):
    return np.concatenate(
        [r["outT"].T / dens[c][:, None] for c, r in enumerate(res.results)], axis=0
    ).astype(np.float32)


def kernel(x, adj, W, a):
    in_maps, dens = _prep_inputs(x, adj, W, a)
    res = _run(in_maps)
    return _finish(res, dens)
